# revision 1
# baseline (speedup 1.0000x reference)
import numpy as np
import jax
import jax.numpy as jnp
from functools import partial
from jax.sharding import Mesh, PartitionSpec as P, NamedSharding

N = 8192
DIM = 64
K = 32
ALPHA = 3.0
M = 8  # cores


def _block(emb1_w, emb2_w, w1, b1, w2, b2, noise_blk, row0):
    # Full [N, dim] node factors are tiny — compute them replicated on
    # every core; only the [N/M, N] adjacency block is per-core work.
    n1 = jnp.tanh(ALPHA * (emb1_w @ w1.T + b1))
    n2 = jnp.tanh(ALPHA * (emb2_w @ w2.T + b2))
    rows = noise_blk.shape[0]
    n1b = jax.lax.dynamic_slice_in_dim(n1, row0, rows, axis=0)
    n2b = jax.lax.dynamic_slice_in_dim(n2, row0, rows, axis=0)
    a = n1b @ n2.T - n2b @ n1.T
    adj = jax.nn.relu(jnp.tanh(ALPHA * a))
    _, t1 = jax.lax.top_k(adj + noise_blk * 0.01, K)
    r = jnp.arange(rows)[:, None]
    mask = jnp.zeros((rows, adj.shape[1]), dtype=adj.dtype).at[r, t1].set(1.0)
    return adj * mask


def _build():
    devs = jax.devices()
    if len(devs) >= M:
        mesh = Mesh(np.array(devs[:M]), ("x",))
        repl = NamedSharding(mesh, P())
        rowsh = NamedSharding(mesh, P("x", None))

        def fn(emb1_w, emb2_w, w1, b1, w2, b2, noise):
            blk = N // M

            def per_shard(e1, e2, w1_, b1_, w2_, b2_, nz):
                i = jax.lax.axis_index("x")
                return _block(e1, e2, w1_, b1_, w2_, b2_, nz, i * blk)

            sm = jax.shard_map(
                per_shard,
                mesh=mesh,
                in_specs=(P(), P(), P(), P(), P(), P(), P("x", None)),
                out_specs=P("x", None),
            )
            return sm(emb1_w, emb2_w, w1, b1, w2, b2, noise)

        jfn = jax.jit(
            fn,
            in_shardings=(repl, repl, repl, repl, repl, repl, rowsh),
            out_shardings=rowsh,
        )
        return jfn, True
    else:
        def fn(emb1_w, emb2_w, w1, b1, w2, b2, noise):
            return _block(emb1_w, emb2_w, w1, b1, w2, b2, noise, 0)
        return jax.jit(fn), False


_JFN = None


def kernel(idx, emb1_w, emb2_w, w1, b1, w2, b2, noise):
    global _JFN
    if _JFN is None:
        _JFN = _build()
    jfn, _ = _JFN
    idx = np.asarray(idx)
    # idx is a permutation gather; apply on host so the device kernel is
    # the pure row-sharded adjacency work.
    e1 = np.asarray(emb1_w, dtype=np.float32)[idx]
    e2 = np.asarray(emb2_w, dtype=np.float32)[idx]
    out = jfn(
        jnp.asarray(e1),
        jnp.asarray(e2),
        jnp.asarray(w1, dtype=jnp.float32),
        jnp.asarray(b1, dtype=jnp.float32),
        jnp.asarray(w2, dtype=jnp.float32),
        jnp.asarray(b2, dtype=jnp.float32),
        jnp.asarray(noise, dtype=jnp.float32),
    )
    return np.asarray(jax.device_get(out), dtype=np.float32)

